# revision 48
# baseline (speedup 1.0000x reference)
"""Trainium2 Bass kernel for nn_CausalAttention (gated-resnet q/k/v projections
+ causal attention). Data-parallel over batch: 8 batches -> 8 NeuronCores.

Per-core computation (batch b), bf16 matmul operands, fp32 accumulation:
  x_q = query[b] (C=256, S=1024)   x_k = key[b] (256, 1024)
  branch(p, x): e+1  = elu(x)+1            (the +1 is folded into next bias:
                h1 = W1 @ (e+1) + b1'      b1' = b1 - rowsum(W1), host-side)
                e1+1 = elu(h1)+1
                h2 = W2 @ (e1+1) + b2' ; a, g = split(h2)
                gr = x + 0.5*(a)*(1+tanh(g/2))
                o  = Wn @ gr               (nin bias == 0 by spec, dropped)
  q = branch(q, x_q); k = branch(k, x_k); v = branch(v, x_k)
  att view: X_att[s, d] = X_cm[s//2, (s%2)*512 + d]  (flat reinterpretation)
  qT_p/kT_p hold one 128-partition slot per head: head n's 64 dims sit at
  partitions 64*(n%2)..64*(n%2)+63 with ZEROS in the other half, so every
  scores matmul is a full-array K=128 (no PE tiling mode, zeros add 0).
  per head n:
    scoresT[s2, s1] = sum_d K_att[s2,d] Q_att[s1,d], 5 psum chunks of <=1024
    eT = exp(scoresT/sqrt(512)) per chunk (ACT); strict-causal mask applied
    on eT by gpsimd affine_select (zero where s1 <= s2 in diagonal blocks)
    PV pieces are issued incrementally right after each chunk's mask so the
    PE stays dense; the augmented-V ones column gives l[s1] in row VS
    final[64n+vs, s1] = outT[vs, s1] / l[s1]  (l[0] patched to 1; per-column
      -half approx-reciprocal + DRAM-bounce broadcast + fused multiply)

All biases are zeros per the problem spec; they are applied only where free
(ACT bias operand / tensor_scalar slot) using host-adjusted values.
"""

import os
import sys
import numpy as np

sys.path.insert(0, "/opt/trn_rl_repo")

C = 256
S = 1024
D = 512
NH = 8
KS = 64
VS = 64
SCALE = 1.0 / float(np.sqrt(512.0))
N_CORES = 8

# eT column layout: 5 psum chunks of <=1024 cols; group j (s2 block j) covers
# s1 in [128j, 1024) and sits at column G[j] + (s1 - 128j).
CHUNK_J = [(0,), (1, 7), (2, 6), (3, 5), (4,)]
CHUNK_BASE = [0, 1024, 2048, 3072, 4096]
CHUNK_LEN = [1024, 1024, 1024, 1024, 512]
G = {}
for _ci, _js in enumerate(CHUNK_J):
    _off = CHUNK_BASE[_ci]
    for _j in _js:
        G[_j] = _off
        _off += S - 128 * _j
    assert _off == CHUNK_BASE[_ci] + CHUNK_LEN[_ci]

CFG = {
    "stop_after": None,   # None | "proj" | "scores"
    "dve_exp_heads": (3,),  # heads with softmax exp as DVE Schraudolph
    "paired_scores": True,  # K=64 row-tiled concurrent head pairs
}


def _bank_pieces(lo, hi):
    """Split [lo, hi) psum column range at 512 boundaries."""
    out = []
    while lo < hi:
        nxt = min(hi, ((lo // 512) + 1) * 512)
        out.append((lo, nxt))
        lo = nxt
    return out


def build_program(cfg=CFG):
    from contextlib import ExitStack

    import concourse.bacc as bacc
    import concourse.bass as bass
    import concourse.tile as tile
    from concourse import mybir
    from concourse.alu_op_type import AluOpType as Op

    f32 = mybir.dt.float32
    mdt = mybir.dt.bfloat16
    i16 = mybir.dt.int16
    AF = mybir.ActivationFunctionType

    nc = bacc.Bacc("TRN2", target_bir_lowering=False, debug=False,
                   num_devices=N_CORES)

    # ---------------- DRAM parameters ----------------
    query = nc.dram_tensor("query", [C, S], mdt, kind="ExternalInput").ap()
    key = nc.dram_tensor("key", [C, S], mdt, kind="ExternalInput").ap()
    wcat = {}
    bcat = {}
    for p in ("q", "k", "v"):
        wcat[p] = nc.dram_tensor(f"{p}_wcat", [C, 1280], mdt, kind="ExternalInput").ap()
        bcat[p] = nc.dram_tensor(f"{p}_bcat", [6 * 128], f32, kind="ExternalInput").ap()
    out_d = nc.dram_tensor("out", [D, S], f32, kind="ExternalOutput").ap()

    with tile.TileContext(nc) as tc, ExitStack() as ctx:
        persist = ctx.enter_context(tc.tile_pool(name="persist", bufs=1))
        dram_pool = ctx.enter_context(tc.tile_pool(name="dram", bufs=1, space="DRAM"))

        # persistent tiles
        xq = persist.tile([128, 2, S], mdt)
        xk = persist.tile([128, 2, S], mdt)
        eluq = persist.tile([128, 2, S], mdt)   # elu(x)+1
        eluk = persist.tile([128, 2, S], mdt)
        # qT_m: [d%128, d//128, s]; kT_z: one 128-partition slot per head with
        # head n's 64 dims at partitions 64*(n%2).. and ZEROS on the other
        # half, so scores matmuls are full-array K=128 (no PE tiling mode).
        # Only the stationary side needs padding: the zero weights gate out
        # the other head's rows of the shared moving operand.
        qT_m = persist.tile([128, 4, S], mdt)
        kT_z = persist.tile([128, NH, S], mdt)
        v_aug = persist.tile([128, 8, NH, VS + 1], mdt)  # [s%128, s//128, n, vs|1]
        tri = persist.tile([128, 128], mdt)     # [k, t2] = 1 if k <= t2
        negeye = persist.tile([128, 128], mdt)  # -1e4 * I

        vproj_dram = dram_pool.tile([D, S], mdt)
        # row-selector constant (row 0 = ones) and the reciprocal staging
        # tile for the in-psum broadcast matmul: rb = E0^T @ rg selects and
        # broadcasts row h of rg across all psum partitions
        e0 = persist.tile([128, 128], mdt)
        e1s = persist.tile([128, 128], mdt)
        rgb = persist.tile([128, 512], mdt)

        warm = persist.tile([128, 512], mdt, name="warm")
        nc.vector.memset(warm, 0.5)

        with ExitStack() as ctx_p:
            pm = ctx_p.enter_context(tc.tile_pool(name="pm", bufs=3, space="PSUM"))
            pnin = ctx_p.enter_context(tc.tile_pool(name="pnin", bufs=2, space="PSUM"))
            work = ctx_p.enter_context(tc.tile_pool(name="wk", bufs=10))

            # PE warm-up. First 4 pairs are a row-tiling concurrency probe
            # (K=64 at partition bases 0/64 -> tiles (0,0)/(64,0)); the rest
            # are plain full-array matmuls.
            wpsA = pnin.tile([128, 512], f32, tag="pn", name="wpsA")
            wpsB = pnin.tile([128, 512], f32, tag="pn", name="wpsB")
            for _ in range(4):
                nc.tensor.matmul(wpsA, lhsT=warm[0:64, 0:128], rhs=warm[0:64, :],
                                 start=True, stop=True)
                nc.tensor.matmul(wpsB, lhsT=warm[64:128, 0:128], rhs=warm[64:128, :],
                                 start=True, stop=True)
            for _ in range(8):
                nc.tensor.matmul(wpsA, lhsT=warm[:, 0:128], rhs=warm,
                                 start=True, stop=True)

            # inputs (k first: the k branch starts the pipeline)
            for cc in range(2):
                nc.sync.dma_start(out=xk[:, cc, :], in_=key[cc * 128:(cc + 1) * 128, :])
            for cc in range(2):
                nc.sync.dma_start(out=xq[:, cc, :], in_=query[cc * 128:(cc + 1) * 128, :])

            # weights + biases (concatenated host-side: 3 DMAs per branch)
            wc = {}
            b1 = {}
            b2ah = {}
            b2gh = {}
            wpool = ctx_p.enter_context(tc.tile_pool(name="wts", bufs=1))
            for p in ("k", "q", "v"):
                wc[p] = wpool.tile([128, 2, 1280], mdt, name=f"wc_{p}")
                for kc in range(2):
                    nc.sync.dma_start(out=wc[p][:, kc, :],
                                      in_=wcat[p][kc * 128:(kc + 1) * 128, :])
                bc = wpool.tile([128, 6], f32, name=f"bc_{p}")
                nc.sync.dma_start(out=bc, in_=bcat[p].rearrange("(x p) -> p x", p=128))
                b1[p] = bc[:, 0:2]
                b2ah[p] = bc[:, 2:4]
                b2gh[p] = bc[:, 4:6]
            w1 = {p: wc[p][:, :, 0:256] for p in wc}
            w2 = {p: wc[p][:, :, 256:768] for p in wc}
            wn = {p: wc[p][:, :, 768:1280] for p in wc}

            # zero the off-half of every kT_z head slot (even heads: parts
            # 64-127, odd heads: parts 0-63); data halves written by nin_T
            nc.gpsimd.memset(kT_z[64:128, 0::2, :], 0.0)
            nc.gpsimd.memset(kT_z[0:64, 1::2, :], 0.0)
            nc.vector.memset(v_aug[:, :, :, VS:VS + 1], 1.0)
            # causal-mask constants for the in-psum mask matmul:
            # tri[k, t2] = 1.0 where t2 - k >= 0 ; negeye = -1e4 on diagonal
            nc.gpsimd.memset(tri, 1.0)
            nc.gpsimd.affine_select(out=tri, in_=tri, compare_op=Op.is_ge,
                                    fill=0.0, base=0, pattern=[[1, 128]],
                                    channel_multiplier=-1)
            nc.gpsimd.memset(negeye, -10000.0)
            nc.gpsimd.affine_select(out=negeye, in_=negeye, compare_op=Op.is_ge,
                                    fill=0.0, base=0, pattern=[[1, 128]],
                                    channel_multiplier=-1)
            nc.gpsimd.affine_select(out=negeye, in_=negeye, compare_op=Op.is_ge,
                                    fill=0.0, base=0, pattern=[[-1, 128]],
                                    channel_multiplier=1)
            nc.gpsimd.memset(e0, 0.0)
            nc.gpsimd.memset(e0[0:1, :], 1.0)
            # e1s: ones on partition 1 only (keep 1 <= p <= 1)
            nc.gpsimd.memset(e1s, 1.0)
            nc.gpsimd.affine_select(out=e1s, in_=e1s, compare_op=Op.is_ge,
                                    fill=0.0, base=-1, pattern=[[0, 128]],
                                    channel_multiplier=1)
            nc.gpsimd.affine_select(out=e1s, in_=e1s, compare_op=Op.is_ge,
                                    fill=0.0, base=1, pattern=[[0, 128]],
                                    channel_multiplier=-1)
            nc.vector.memset(rgb, 0.0)

            def elu1_psum(dst, ps, bias_ap):
                """dst = elu(ps + b)+1 for PSUM ps (2 DVE + 1 ACT)."""
                r = work.tile([128, S], mdt, tag="wk")
                e = work.tile([128, S], mdt, tag="wk")
                nc.vector.tensor_scalar(r, ps, bias_ap, 0.0, Op.add, Op.max)
                nc.scalar.activation(e, ps, AF.Exp, bias=bias_ap)
                nc.vector.scalar_tensor_tensor(dst, e, 1.0, r, Op.min, Op.add)

            def elu1_in2(dst3, src3):
                """dst = elu(src)+1 over the full [128, 2S] tile; one big ACT
                exp, per-half DVE combine."""
                e2 = work.tile([128, 2, S], mdt, tag="wke", bufs=2, name="e2")
                nc.scalar.activation(e2.rearrange("p a b -> p (a b)"),
                                     src3.rearrange("p a b -> p (a b)"), AF.Exp)
                for cc in range(2):
                    r = work.tile([128, S], mdt, tag="wk")
                    nc.vector.tensor_scalar(r, src3[:, cc, :], 0.0, 0.0,
                                            Op.max, Op.add)
                    nc.vector.scalar_tensor_tensor(dst3[:, cc, :], e2[:, cc, :],
                                                   1.0, r, Op.min, Op.add)

            elu1_in2(eluk, xk)
            elu1_in2(eluq, xq)

            src_of = {"q": (xq, eluq), "k": (xk, eluk), "v": (xk, eluk)}
            BRS = ("k", "q", "v")

            # ---- h1 + e1 (interleaved across branches for PE overlap) ----
            e1 = {}
            for p in BRS:
                e1[p] = work.tile([128, 2, S], mdt, tag=f"e1_{p}", bufs=1,
                                  name=f"e1_{p}")
            for p in BRS:
                elu_in = src_of[p][1]
                for mc in range(2):
                    ps = pm.tile([128, 1024], f32, tag="pm")
                    for nk in range(2):
                        for kc in range(2):
                            nc.tensor.matmul(
                                ps[:, nk * 512:(nk + 1) * 512],
                                lhsT=w1[p][:, kc, mc * 128:(mc + 1) * 128],
                                rhs=elu_in[:, kc, nk * 512:(nk + 1) * 512],
                                start=(kc == 0), stop=(kc == 1))
                    elu1_psum(e1[p][:, mc, :], ps, b1[p][:, mc:mc + 1])

            # ---- h2 + GLU -> gr ----
            gr = {}
            for p in BRS:
                gr[p] = work.tile([128, 2, S], mdt, tag=f"gr_{p}", bufs=1,
                                  name=f"gr_{p}")
            for p in BRS:
                x3 = src_of[p][0]
                for cc in range(2):
                    ps_a = pm.tile([128, 1024], f32, tag="pm")
                    for nk in range(2):
                        for kc in range(2):
                            nc.tensor.matmul(
                                ps_a[:, nk * 512:(nk + 1) * 512],
                                lhsT=w2[p][:, kc, cc * 128:(cc + 1) * 128],
                                rhs=e1[p][:, kc, nk * 512:(nk + 1) * 512],
                                start=(kc == 0), stop=(kc == 1))
                    ps_g = pm.tile([128, 1024], f32, tag="pm")
                    for nk in range(2):
                        for kc in range(2):
                            nc.tensor.matmul(
                                ps_g[:, nk * 512:(nk + 1) * 512],
                                lhsT=w2[p][:, kc, (2 + cc) * 128:(3 + cc) * 128],
                                rhs=e1[p][:, kc, nk * 512:(nk + 1) * 512],
                                start=(kc == 0), stop=(kc == 1))
                    ha = work.tile([128, S], mdt, tag="wk")
                    tg = work.tile([128, S], mdt, tag="wk")
                    u = work.tile([128, S], mdt, tag="wk")
                    nc.scalar.activation(ha, ps_a, AF.Identity,
                                         bias=b2ah[p][:, cc:cc + 1], scale=0.5)
                    nc.scalar.activation(tg, ps_g, AF.Tanh,
                                         bias=b2gh[p][:, cc:cc + 1], scale=0.5)
                    nc.vector.scalar_tensor_tensor(u, tg, 1.0, ha, Op.add, Op.mult)
                    nc.gpsimd.tensor_tensor(gr[p][:, cc, :], u, x3[:, cc, :], Op.add)

            # ---- nin: k (transposed), q (transposed), v (channel-major) ----
            def nin_T(p):
                for hw_p in (0, 4, 1, 5, 2, 6, 3, 7):
                    ps = pnin.tile([128, 512], f32, tag="pn")
                    for kc in range(2):
                        nc.tensor.matmul(
                            ps,
                            lhsT=gr[p][:, kc, hw_p * 128:(hw_p + 1) * 128],
                            rhs=wn[p][:, kc, :],
                            start=(kc == 0), stop=(kc == 1))
                    tp, jj = hw_p % 4, hw_p // 4
                    if p == "q":
                        nc.scalar.activation(qT_m[:, tp, jj::2], ps, AF.Identity)
                    elif jj == 0:
                        nc.scalar.activation(kT_z[0:64, 2 * tp, jj::2],
                                             ps[0:64, :], AF.Identity)
                        nc.scalar.activation(kT_z[64:128, 2 * tp + 1, jj::2],
                                             ps[64:128, :], AF.Identity)
                    else:
                        nc.vector.tensor_copy(kT_z[0:64, 2 * tp, jj::2],
                                              ps[0:64, :])
                        nc.vector.tensor_copy(kT_z[64:128, 2 * tp + 1, jj::2],
                                              ps[64:128, :])

            def nin_v():
                v_sb = work.tile([128, 4, S], mdt, tag="vsb", bufs=1)
                for mc in range(4):
                    ps = pm.tile([128, 1024], f32, tag="pm")
                    for nk in range(2):
                        for kc in range(2):
                            nc.tensor.matmul(
                                ps[:, nk * 512:(nk + 1) * 512],
                                lhsT=wn["v"][:, kc, mc * 128:(mc + 1) * 128],
                                rhs=gr["v"][:, kc, nk * 512:(nk + 1) * 512],
                                start=(kc == 0), stop=(kc == 1))
                    nc.scalar.activation(v_sb[:, mc, :], ps, AF.Identity)
                    nc.sync.dma_start(out=vproj_dram[mc * 128:(mc + 1) * 128, :],
                                      in_=v_sb[:, mc, :])
                # v_aug[p2, j, n, u] = V_att[128j+p2, 64n+u]
                for j in range(8):
                    src = vproj_dram[64 * j:64 * j + 64, :]
                    src = src.rearrange("c (h n u) -> c h n u", h=2, n=NH)
                    nc.sync.dma_start(out=v_aug[:, j, :, 0:VS], in_=src)

            nin_T("k")
            nin_T("q")
            nin_v()

        # ---------------- attention ----------------
        stop_after = cfg.get("stop_after")
        dve_heads = set(cfg.get("dve_exp_heads", ()))
        # Schraudolph exp constants for bf16-bit output via int16:
        # bits = round(x*SCALE*(2^7/ln2) + (127*2^7 - 5.76))
        SCH_A = float(SCALE * 128.0 / np.log(2.0))
        SCH_B = 16250.24

        if stop_after == "proj":
            fin0 = persist.tile([128, S], f32)
            nc.vector.tensor_copy(fin0, qT_m[:, 0, :])
            nc.sync.dma_start(out=out_d[0:128, :], in_=fin0)
            nc.vector.tensor_copy(fin0, kT_z[:, 1, :])
            nc.sync.dma_start(out=out_d[128:256, :], in_=fin0)
            nc.vector.tensor_copy(fin0, v_aug.rearrange("p a b c -> p (a b c)")[:, 0:S])
            nc.sync.dma_start(out=out_d[256:384, :], in_=fin0)
            nc.sync.dma_start(out=out_d[384:512, :], in_=fin0)

        with ExitStack() as ctx_a:
            scp = ctx_a.enter_context(tc.tile_pool(name="scp", bufs=2, space="PSUM"))
            pvp = ctx_a.enter_context(tc.tile_pool(name="pvp", bufs=4, space="PSUM"))
            eT_pool = ctx_a.enter_context(tc.tile_pool(name="eT", bufs=4))
            epi = ctx_a.enter_context(tc.tile_pool(name="epi", bufs=2))

            import concourse.bass as bass_mod

            for m in range(4 if stop_after != "proj" else 0):
                n0, n1 = 2 * m, 2 * m + 1
                eT = {n0: eT_pool.tile([128, 4608], mdt, tag="eT", name="eT0"),
                      n1: eT_pool.tile([128, 4608], mdt, tag="eT", name="eT1")}
                pvt = {}
                ul = epi.tile([65, 4, 512], mdt, tag="ul")

                def epilogue_c(c):
                    lg = epi.tile([2, 512], mdt, tag="lg")
                    nc.sync.dma_start(out=lg, in_=ul[64:65, 2 * c:2 * c + 2, :])
                    if c == 0:
                        nc.vector.memset(lg[0:2, 0:1], 1.0)  # l[s1=0] == 0 -> 1
                    lgf = epi.tile([2, 512], f32, tag="lgf")
                    rgf = epi.tile([2, 512], f32, tag="rgf")
                    nc.vector.tensor_copy(lgf, lg)
                    nc.vector.reciprocal_approx_fast(out=rgf, in_=lgf)
                    nc.vector.tensor_copy(rgb[0:2, :], rgf)
                    for h, n in enumerate((n0, n1)):
                        # rb[p, s] = rgb[h, s] for all p (row-selector matmul)
                        rb = pvp.tile([128, 512], f32, tag="pv", name="rb")
                        nc.tensor.matmul(rb, lhsT=(e0 if h == 0 else e1s),
                                         rhs=rgb, start=True, stop=True)
                        fin = epi.tile([64, 512], f32, tag="fin")
                        nc.vector.tensor_tensor(fin, ul[0:64, 2 * c + h, :],
                                                rb[0:64, :], Op.mult)
                        eng = nc.sync if c else nc.scalar
                        eng.dma_start(out=out_d[VS * n:VS * (n + 1),
                                                512 * c:512 * (c + 1)], in_=fin)

                for ci, js in enumerate(CHUNK_J):
                    clen = CHUNK_LEN[ci]
                    cbase = CHUNK_BASE[ci]
                    ps = {n0: scp.tile([128, 1024], f32, tag="sc", name="ps0"),
                          n1: scp.tile([128, 1024], f32, tag="sc", name="ps1")}
                    for j in js:
                        base = G[j] - cbase
                        for lo, hi in _bank_pieces(base, base + S - 128 * j):
                            s1a = 128 * j + (lo - base)
                            s1b = 128 * j + (hi - base)
                            diag = (lo == base)
                            if cfg.get("paired_scores"):
                                # concurrent row-tiles: head n at partition
                                # half 64*(n%2) -> tile (0,0) / (64,0)
                                for n in (n0, n1):
                                    po = 64 * (n % 2)
                                    nc.tensor.matmul(
                                        ps[n][:, lo:hi],
                                        lhsT=kT_z[po:po + 64, n,
                                                  128 * j:128 * (j + 1)],
                                        rhs=qT_m[po:po + 64, m, s1a:s1b],
                                        start=True, stop=not diag)
                            else:
                                for n in (n0, n1):
                                    nc.tensor.matmul(
                                        ps[n][:, lo:hi],
                                        lhsT=kT_z[:, n, 128 * j:128 * (j + 1)],
                                        rhs=qT_m[:, m, s1a:s1b],
                                        start=True, stop=not diag)
                            if diag:
                                for n in (n0, n1):
                                    nc.tensor.matmul(
                                        ps[n][:, base:base + 128],
                                        lhsT=tri, rhs=negeye,
                                        start=False, stop=True)
                    for n in (n0, n1):
                        if n in dve_heads:
                            dst = eT[n][:, cbase:cbase + clen].bitcast(i16)
                            nc.vector.tensor_scalar(dst, ps[n][:, 0:clen],
                                                    SCH_A, SCH_B, Op.mult, Op.add)
                        else:
                            nc.scalar.activation(eT[n][:, cbase:cbase + clen],
                                                 ps[n][:, 0:clen], AF.Exp, scale=SCALE)

                    if stop_after == "scores":
                        continue
                    # incremental PV: issue pieces for every j in this chunk
                    for j in js:
                        for n in (n0, n1):
                            for c in range(2):
                                if j > 4 * c + 3:
                                    continue
                                if j == 0:
                                    pvt[n, c] = pvp.tile([128, 512], f32, tag="pv",
                                                         name=f"pv{n % 2}{c}")
                                s1a = max(512 * c, 128 * j)
                                s1b = 512 * (c + 1)
                                last = (c == 0 and j == 3) or (c == 1 and j == 4)
                                nc.tensor.matmul(
                                    pvt[n, c][0:65, s1a - 512 * c:512],
                                    lhsT=v_aug[:, j, n, :],
                                    rhs=eT[n][:, G[j] + (s1a - 128 * j):G[j] + (s1b - 128 * j)],
                                    start=(j == 0), stop=last)
                                if last:
                                    nc.vector.tensor_copy(
                                        ul[:, 2 * c + (n - n0), :], pvt[n, c][0:65, :])
                    if stop_after is None:
                        if ci == 3:
                            epilogue_c(0)
                        elif ci == 4:
                            epilogue_c(1)

                if stop_after == "scores":
                    fin1 = epi.tile([128, 512], f32, tag="fin1")
                    for n in (n0, n1):
                        nc.vector.tensor_copy(fin1, eT[n][:, 0:512])
                        nc.sync.dma_start(
                            out=out_d[64 * (n // 2):64 * (n // 2) + 128,
                                      512 * (n % 2):512 * (n % 2) + 512],
                            in_=fin1)
                    continue

    nc.compile()
    return nc


_CACHE = {}


def _get_program(cfg_key=None):
    key = cfg_key or "default"
    if key not in _CACHE:
        _CACHE[key] = build_program(CFG)
    return _CACHE[key]


def make_in_map(inp, b):
    """Per-core input dict for batch b (weights host-transposed/cast to bf16;
    biases host-adjusted for the elu(x)+1 formulation)."""
    import ml_dtypes
    wt = np.dtype(ml_dtypes.bfloat16)
    m = {
        "query": np.ascontiguousarray(inp["query"][b].reshape(C, S)).astype(wt),
        "key": np.ascontiguousarray(inp["key"][b].reshape(C, S)).astype(wt),
    }
    for p in ("q", "k", "v"):
        w1 = inp[f"{p}_gr_w1"]
        w2 = inp[f"{p}_gr_w2"]
        m[f"{p}_wcat"] = np.ascontiguousarray(np.concatenate(
            [w1.T, w2.T, inp[f"{p}_nin_w"].T], axis=1)).astype(wt)
        b1_eff = inp[f"{p}_gr_b1"] - w1.sum(axis=1)
        b2_eff = inp[f"{p}_gr_b2"] - w2.sum(axis=1)
        m[f"{p}_bcat"] = np.concatenate(
            [b1_eff, 0.5 * b2_eff[:C], 0.5 * b2_eff[C:]]).astype(np.float32)
    return m


def kernel(**inputs):
    from concourse.bass_utils import run_bass_kernel_spmd

    nc = _get_program()
    inp = {k: np.asarray(v, dtype=np.float32) for k, v in inputs.items()}

    in_maps = [make_in_map(inp, b) for b in range(N_CORES)]

    trace = bool(int(os.environ.get("BASS_KERNEL_TRACE", "0")))
    res = run_bass_kernel_spmd(nc, in_maps, core_ids=list(range(N_CORES)),
                               trace=trace)
    LAST_RUN["exec_time_ns"] = getattr(res, "exec_time_ns", None)
    LAST_RUN["results"] = res
    out = np.stack([res.results[i]["out"].reshape(D, 32, 32)
                    for i in range(N_CORES)])
    return out.astype(np.float32)


LAST_RUN = {}


if __name__ == "__main__":
    nc = build_program()
    print("compiled OK")


# revision 58
# speedup vs baseline: 1.1763x; 1.1763x over previous
"""Trainium2 Bass kernel for nn_CausalAttention (gated-resnet q/k/v projections
+ causal attention). Data-parallel over batch: 8 batches -> 8 NeuronCores.

Per-core computation (batch b), bf16 matmul operands, fp32 accumulation:
  x_q = query[b] (C=256, S=1024)   x_k = key[b] (256, 1024)
  branch(p, x): e+1  = elu(x)+1            (the +1 is folded into next bias:
                h1 = W1 @ (e+1) + b1'      b1' = b1 - rowsum(W1), host-side)
                e1+1 = elu(h1)+1
                h2 = W2 @ (e1+1) + b2' ; a, g = split(h2)
                gr = x + 0.5*(a)*(1+tanh(g/2))
                o  = Wn @ gr               (nin bias == 0 by spec, dropped)
  q = branch(q, x_q); k = branch(k, x_k); v = branch(v, x_k)
  att view: X_att[s, d] = X_cm[s//2, (s%2)*512 + d]  (flat reinterpretation)
  qT_p/kT_p hold one 128-partition slot per head: head n's 64 dims sit at
  partitions 64*(n%2)..64*(n%2)+63 with ZEROS in the other half, so every
  scores matmul is a full-array K=128 (no PE tiling mode, zeros add 0).
  per head n:
    scoresT[s2, s1] = sum_d K_att[s2,d] Q_att[s1,d], 5 psum chunks of <=1024
    eT = exp(scoresT/sqrt(512)) per chunk (ACT); strict-causal mask applied
    on eT by gpsimd affine_select (zero where s1 <= s2 in diagonal blocks)
    PV pieces are issued incrementally right after each chunk's mask so the
    PE stays dense; the augmented-V ones column gives l[s1] in row VS
    final[64n+vs, s1] = outT[vs, s1] / l[s1]  (l[0] patched to 1; per-column
      -half approx-reciprocal + DRAM-bounce broadcast + fused multiply)

All biases are zeros per the problem spec; they are applied only where free
(ACT bias operand / tensor_scalar slot) using host-adjusted values.
"""

import os
import sys
import numpy as np

sys.path.insert(0, "/opt/trn_rl_repo")

C = 256
S = 1024
D = 512
NH = 8
KS = 64
VS = 64
SCALE = 1.0 / float(np.sqrt(512.0))
N_CORES = 8

# eT column layout: 5 psum chunks of <=1024 cols; group j (s2 block j) covers
# s1 in [128j, 1024) and sits at column G[j] + (s1 - 128j).
CHUNK_J = [(0,), (1, 7), (2, 6), (3, 5), (4,)]
CHUNK_BASE = [0, 1024, 2048, 3072, 4096]
CHUNK_LEN = [1024, 1024, 1024, 1024, 512]
G = {}
for _ci, _js in enumerate(CHUNK_J):
    _off = CHUNK_BASE[_ci]
    for _j in _js:
        G[_j] = _off
        _off += S - 128 * _j
    assert _off == CHUNK_BASE[_ci] + CHUNK_LEN[_ci]

CFG = {
    "stop_after": None,   # None | "proj" | "scores"
    "dve_exp_heads": (3, 5),  # heads with softmax exp as DVE Schraudolph
    "paired_scores": False,  # K=64 row-tiled concurrent head pairs
}


def _bank_pieces(lo, hi):
    """Split [lo, hi) psum column range at 512 boundaries."""
    out = []
    while lo < hi:
        nxt = min(hi, ((lo // 512) + 1) * 512)
        out.append((lo, nxt))
        lo = nxt
    return out


def build_program(cfg=CFG):
    from contextlib import ExitStack

    import concourse.bacc as bacc
    import concourse.bass as bass
    import concourse.tile as tile
    from concourse import mybir
    from concourse.alu_op_type import AluOpType as Op

    f32 = mybir.dt.float32
    mdt = mybir.dt.bfloat16
    i16 = mybir.dt.int16
    AF = mybir.ActivationFunctionType

    nc = bacc.Bacc("TRN2", target_bir_lowering=False, debug=False,
                   num_devices=N_CORES)

    # ---------------- DRAM parameters ----------------
    query = nc.dram_tensor("query", [C, S], mdt, kind="ExternalInput").ap()
    key = nc.dram_tensor("key", [C, S], mdt, kind="ExternalInput").ap()
    wcat = {}
    bcat = {}
    for p in ("q", "k", "v"):
        wcat[p] = nc.dram_tensor(f"{p}_wcat", [C, 1280], mdt, kind="ExternalInput").ap()
        bcat[p] = nc.dram_tensor(f"{p}_bcat", [8 * 128], f32, kind="ExternalInput").ap()
    out_d = nc.dram_tensor("out", [D, S], f32, kind="ExternalOutput").ap()

    with tile.TileContext(nc) as tc, ExitStack() as ctx:
        persist = ctx.enter_context(tc.tile_pool(name="persist", bufs=1))
        dram_pool = ctx.enter_context(tc.tile_pool(name="dram", bufs=1, space="DRAM"))

        # persistent tiles
        xq = persist.tile([128, 2, S], mdt)
        xk = persist.tile([128, 2, S], mdt)
        eluq = persist.tile([128, 2, S], mdt)   # elu(x)+1
        eluk = persist.tile([128, 2, S], mdt)
        # qT_m: [d%128, d//128, s]; kT_z: one 128-partition slot per head with
        # head n's 64 dims at partitions 64*(n%2).. and ZEROS on the other
        # half, so scores matmuls are full-array K=128 (no PE tiling mode).
        # Only the stationary side needs padding: the zero weights gate out
        # the other head's rows of the shared moving operand.
        qT_m = persist.tile([128, 4, S], mdt)
        kT_z = persist.tile([128, NH, S], mdt)
        v_aug = persist.tile([128, 8, NH, VS + 1], mdt)  # [s%128, s//128, n, vs|1]
        tri = persist.tile([128, 128], mdt)     # [k, t2] = 1 if k <= t2
        negeye = persist.tile([128, 128], mdt)  # -1e4 * I

        vproj_dram = dram_pool.tile([D, S], mdt)
        # row-selector constant (row 0 = ones) and the reciprocal staging
        # tile for the in-psum broadcast matmul: rb = E0^T @ rg selects and
        # broadcasts row h of rg across all psum partitions
        e0 = persist.tile([128, 128], mdt)
        e1s = persist.tile([128, 128], mdt)
        rgb = persist.tile([128, 512], mdt)

        warm = persist.tile([128, 512], mdt, name="warm")
        nc.vector.memset(warm, 0.5)

        with ExitStack() as ctx_p:
            pm = ctx_p.enter_context(tc.tile_pool(name="pm", bufs=3, space="PSUM"))
            pnin = ctx_p.enter_context(tc.tile_pool(name="pnin", bufs=2, space="PSUM"))
            work = ctx_p.enter_context(tc.tile_pool(name="wk", bufs=10))

            # PE warm-up. First 4 pairs are a row-tiling concurrency probe
            # (K=64 at partition bases 0/64 -> tiles (0,0)/(64,0)); the rest
            # are plain full-array matmuls.
            wpsA = pnin.tile([128, 512], f32, tag="pn", name="wpsA")
            wpsB = pnin.tile([128, 512], f32, tag="pn", name="wpsB")
            for _ in range(4):
                nc.tensor.matmul(wpsA, lhsT=warm[0:64, 0:128], rhs=warm[0:64, :],
                                 start=True, stop=True)
                nc.tensor.matmul(wpsB, lhsT=warm[64:128, 0:128], rhs=warm[64:128, :],
                                 start=True, stop=True)
            for _ in range(8):
                nc.tensor.matmul(wpsA, lhsT=warm[:, 0:128], rhs=warm,
                                 start=True, stop=True)

            # inputs (k first: the k branch starts the pipeline)
            for cc in range(2):
                nc.sync.dma_start(out=xk[:, cc, :], in_=key[cc * 128:(cc + 1) * 128, :])
            for cc in range(2):
                nc.sync.dma_start(out=xq[:, cc, :], in_=query[cc * 128:(cc + 1) * 128, :])

            # weights + biases (concatenated host-side: 3 DMAs per branch)
            wc = {}
            b1 = {}
            b2ah = {}
            b2gh = {}
            b1p1 = {}
            wpool = ctx_p.enter_context(tc.tile_pool(name="wts", bufs=1))
            for p in ("k", "q", "v"):
                wc[p] = wpool.tile([128, 2, 1280], mdt, name=f"wc_{p}")
                for kc in range(2):
                    nc.sync.dma_start(out=wc[p][:, kc, :],
                                      in_=wcat[p][kc * 128:(kc + 1) * 128, :])
                bc = wpool.tile([128, 8], f32, name=f"bc_{p}")
                nc.sync.dma_start(out=bc, in_=bcat[p].rearrange("(x p) -> p x", p=128))
                b1[p] = bc[:, 0:2]
                b2ah[p] = bc[:, 2:4]
                b2gh[p] = bc[:, 4:6]
                b1p1[p] = bc[:, 6:8]   # b1 + 1 (for elu+1 = min(exp, relu+1))
            w1 = {p: wc[p][:, :, 0:256] for p in wc}
            w2 = {p: wc[p][:, :, 256:768] for p in wc}
            wn = {p: wc[p][:, :, 768:1280] for p in wc}

            # zero the off-half of every kT_z head slot (even heads: parts
            # 64-127, odd heads: parts 0-63); data halves written by nin_T
            nc.gpsimd.memset(kT_z[64:128, 0::2, :], 0.0)
            nc.gpsimd.memset(kT_z[0:64, 1::2, :], 0.0)
            nc.vector.memset(v_aug[:, :, :, VS:VS + 1], 1.0)
            # causal-mask constants for the in-psum mask matmul:
            # tri[k, t2] = 1.0 where t2 - k >= 0 ; negeye = -1e4 on diagonal
            nc.gpsimd.memset(tri, 1.0)
            nc.gpsimd.affine_select(out=tri, in_=tri, compare_op=Op.is_ge,
                                    fill=0.0, base=0, pattern=[[1, 128]],
                                    channel_multiplier=-1)
            nc.gpsimd.memset(negeye, -10000.0)
            nc.gpsimd.affine_select(out=negeye, in_=negeye, compare_op=Op.is_ge,
                                    fill=0.0, base=0, pattern=[[1, 128]],
                                    channel_multiplier=-1)
            nc.gpsimd.affine_select(out=negeye, in_=negeye, compare_op=Op.is_ge,
                                    fill=0.0, base=0, pattern=[[-1, 128]],
                                    channel_multiplier=1)
            nc.gpsimd.memset(e0, 0.0)
            nc.gpsimd.memset(e0[0:1, :], 1.0)
            # e1s: ones on partition 1 only (keep 1 <= p <= 1)
            nc.gpsimd.memset(e1s, 1.0)
            nc.gpsimd.affine_select(out=e1s, in_=e1s, compare_op=Op.is_ge,
                                    fill=0.0, base=-1, pattern=[[0, 128]],
                                    channel_multiplier=1)
            nc.gpsimd.affine_select(out=e1s, in_=e1s, compare_op=Op.is_ge,
                                    fill=0.0, base=1, pattern=[[0, 128]],
                                    channel_multiplier=-1)
            nc.vector.memset(rgb, 0.0)

            def elu1_psum(dst, ps, bias_ap, bias1_ap):
                """dst = elu(ps+b)+1 = min(exp(ps+b), relu(ps+b)+1); the
                relu+1 is max(ps+b+1, 1) so it fits one tensor_scalar."""
                r = work.tile([128, S], mdt, tag="wk")
                e = work.tile([128, S], mdt, tag="wk")
                nc.vector.tensor_scalar(r, ps, bias1_ap, 1.0, Op.add, Op.max)
                nc.scalar.activation(e, ps, AF.Exp, bias=bias_ap)
                nc.vector.tensor_tensor(dst, e, r, Op.min)

            def elu1_in2(dst3, src3):
                """dst = elu(src)+1 over the full [128, 2S] tile; one big ACT
                exp, per-half DVE combine."""
                e2 = work.tile([128, 2, S], mdt, tag="wke", bufs=2, name="e2")
                nc.scalar.activation(e2.rearrange("p a b -> p (a b)"),
                                     src3.rearrange("p a b -> p (a b)"), AF.Exp)
                for cc in range(2):
                    r = work.tile([128, S], mdt, tag="wk")
                    nc.vector.tensor_scalar(r, src3[:, cc, :], 0.0, 1.0,
                                            Op.max, Op.add)
                    nc.vector.tensor_tensor(dst3[:, cc, :], e2[:, cc, :],
                                            r, Op.min)

            elu1_in2(eluk, xk)
            elu1_in2(eluq, xq)

            src_of = {"q": (xq, eluq), "k": (xk, eluk), "v": (xk, eluk)}
            BRS = ("k", "q", "v")

            # ---- h1 + e1 (interleaved across branches for PE overlap) ----
            e1 = {}
            for p in BRS:
                e1[p] = work.tile([128, 2, S], mdt, tag=f"e1_{p}", bufs=1,
                                  name=f"e1_{p}")
            for p in BRS:
                elu_in = src_of[p][1]
                for mc in range(2):
                    ps = pm.tile([128, 1024], f32, tag="pm")
                    for nk in range(2):
                        for kc in range(2):
                            nc.tensor.matmul(
                                ps[:, nk * 512:(nk + 1) * 512],
                                lhsT=w1[p][:, kc, mc * 128:(mc + 1) * 128],
                                rhs=elu_in[:, kc, nk * 512:(nk + 1) * 512],
                                start=(kc == 0), stop=(kc == 1))
                    elu1_psum(e1[p][:, mc, :], ps, b1[p][:, mc:mc + 1],
                              b1p1[p][:, mc:mc + 1])

            # ---- h2 + GLU -> gr ----
            gr = {}
            for p in BRS:
                gr[p] = work.tile([128, 2, S], mdt, tag=f"gr_{p}", bufs=1,
                                  name=f"gr_{p}")
            for p in BRS:
                x3 = src_of[p][0]
                for cc in range(2):
                    ps_a = pm.tile([128, 1024], f32, tag="pm")
                    for nk in range(2):
                        for kc in range(2):
                            nc.tensor.matmul(
                                ps_a[:, nk * 512:(nk + 1) * 512],
                                lhsT=w2[p][:, kc, cc * 128:(cc + 1) * 128],
                                rhs=e1[p][:, kc, nk * 512:(nk + 1) * 512],
                                start=(kc == 0), stop=(kc == 1))
                    ps_g = pm.tile([128, 1024], f32, tag="pm")
                    for nk in range(2):
                        for kc in range(2):
                            nc.tensor.matmul(
                                ps_g[:, nk * 512:(nk + 1) * 512],
                                lhsT=w2[p][:, kc, (2 + cc) * 128:(3 + cc) * 128],
                                rhs=e1[p][:, kc, nk * 512:(nk + 1) * 512],
                                start=(kc == 0), stop=(kc == 1))
                    ha = work.tile([128, S], mdt, tag="wk")
                    tg = work.tile([128, S], mdt, tag="wk")
                    u = work.tile([128, S], mdt, tag="wk")
                    nc.scalar.activation(ha, ps_a, AF.Identity,
                                         bias=b2ah[p][:, cc:cc + 1], scale=0.5)
                    nc.scalar.activation(tg, ps_g, AF.Tanh,
                                         bias=b2gh[p][:, cc:cc + 1], scale=0.5)
                    nc.vector.scalar_tensor_tensor(u, tg, 1.0, ha, Op.add, Op.mult)
                    nc.gpsimd.tensor_tensor(gr[p][:, cc, :], u, x3[:, cc, :], Op.add)

            # ---- nin: k (transposed), q (transposed), v (channel-major) ----
            def nin_T(p):
                for hw_p in (0, 4, 1, 5, 2, 6, 3, 7):
                    ps = pnin.tile([128, 512], f32, tag="pn")
                    for kc in range(2):
                        nc.tensor.matmul(
                            ps,
                            lhsT=gr[p][:, kc, hw_p * 128:(hw_p + 1) * 128],
                            rhs=wn[p][:, kc, :],
                            start=(kc == 0), stop=(kc == 1))
                    tp, jj = hw_p % 4, hw_p // 4
                    if p == "q":
                        nc.scalar.activation(qT_m[:, tp, jj::2], ps, AF.Identity)
                    elif jj == 0:
                        nc.scalar.activation(kT_z[0:64, 2 * tp, jj::2],
                                             ps[0:64, :], AF.Identity)
                        nc.scalar.activation(kT_z[64:128, 2 * tp + 1, jj::2],
                                             ps[64:128, :], AF.Identity)
                    else:
                        nc.vector.tensor_copy(kT_z[0:64, 2 * tp, jj::2],
                                              ps[0:64, :])
                        nc.vector.tensor_copy(kT_z[64:128, 2 * tp + 1, jj::2],
                                              ps[64:128, :])

            def nin_v():
                v_sb = work.tile([128, 4, S], mdt, tag="vsb", bufs=1)
                for mc in range(4):
                    ps = pm.tile([128, 1024], f32, tag="pm")
                    for nk in range(2):
                        for kc in range(2):
                            nc.tensor.matmul(
                                ps[:, nk * 512:(nk + 1) * 512],
                                lhsT=wn["v"][:, kc, mc * 128:(mc + 1) * 128],
                                rhs=gr["v"][:, kc, nk * 512:(nk + 1) * 512],
                                start=(kc == 0), stop=(kc == 1))
                    nc.scalar.activation(v_sb[:, mc, :], ps, AF.Identity)
                    nc.sync.dma_start(out=vproj_dram[mc * 128:(mc + 1) * 128, :],
                                      in_=v_sb[:, mc, :])
                # v_aug[p2, j, n, u] = V_att[128j+p2, 64n+u]
                for j in range(8):
                    src = vproj_dram[64 * j:64 * j + 64, :]
                    src = src.rearrange("c (h n u) -> c h n u", h=2, n=NH)
                    nc.sync.dma_start(out=v_aug[:, j, :, 0:VS], in_=src)

            nin_T("k")
            nin_T("q")
            nin_v()

        # ---------------- attention ----------------
        stop_after = cfg.get("stop_after")
        dve_heads = set(cfg.get("dve_exp_heads", ()))
        # Schraudolph exp constants for bf16-bit output via int16:
        # bits = round(x*SCALE*(2^7/ln2) + (127*2^7 - 5.76))
        SCH_A = float(SCALE * 128.0 / np.log(2.0))
        SCH_B = 16250.24

        if stop_after == "proj":
            fin0 = persist.tile([128, S], f32)
            nc.vector.tensor_copy(fin0, qT_m[:, 0, :])
            nc.sync.dma_start(out=out_d[0:128, :], in_=fin0)
            nc.vector.tensor_copy(fin0, kT_z[:, 1, :])
            nc.sync.dma_start(out=out_d[128:256, :], in_=fin0)
            nc.vector.tensor_copy(fin0, v_aug.rearrange("p a b c -> p (a b c)")[:, 0:S])
            nc.sync.dma_start(out=out_d[256:384, :], in_=fin0)
            nc.sync.dma_start(out=out_d[384:512, :], in_=fin0)

        with ExitStack() as ctx_a:
            scp = ctx_a.enter_context(tc.tile_pool(name="scp", bufs=2, space="PSUM"))
            pvp = ctx_a.enter_context(tc.tile_pool(name="pvp", bufs=4, space="PSUM"))
            eT_pool = ctx_a.enter_context(tc.tile_pool(name="eT", bufs=4))
            epi = ctx_a.enter_context(tc.tile_pool(name="epi", bufs=2))

            import concourse.bass as bass_mod

            for m in range(4 if stop_after != "proj" else 0):
                n0, n1 = 2 * m, 2 * m + 1
                eT = {n0: eT_pool.tile([128, 4608], mdt, tag="eT", name="eT0"),
                      n1: eT_pool.tile([128, 4608], mdt, tag="eT", name="eT1")}
                pvt = {}
                ul = epi.tile([65, 4, 512], mdt, tag="ul")

                def epilogue_c(c):
                    lg = epi.tile([2, 512], mdt, tag="lg")
                    nc.sync.dma_start(out=lg, in_=ul[64:65, 2 * c:2 * c + 2, :])
                    if c == 0:
                        nc.vector.memset(lg[0:2, 0:1], 1.0)  # l[s1=0] == 0 -> 1
                    lgf = epi.tile([2, 512], f32, tag="lgf")
                    rgf = epi.tile([2, 512], f32, tag="rgf")
                    nc.vector.tensor_copy(lgf, lg)
                    nc.vector.reciprocal_approx_fast(out=rgf, in_=lgf)
                    nc.vector.tensor_copy(rgb[0:2, :], rgf)
                    for h, n in enumerate((n0, n1)):
                        # rb[p, s] = rgb[h, s] for all p (row-selector matmul)
                        rb = pvp.tile([128, 512], f32, tag="pv", name="rb")
                        nc.tensor.matmul(rb, lhsT=(e0 if h == 0 else e1s),
                                         rhs=rgb, start=True, stop=True)
                        fin = epi.tile([64, 512], f32, tag="fin")
                        nc.vector.tensor_tensor(fin, ul[0:64, 2 * c + h, :],
                                                rb[0:64, :], Op.mult)
                        nc.sync.dma_start(out=out_d[VS * n:VS * (n + 1),
                                                    512 * c:512 * (c + 1)], in_=fin)

                for ci, js in enumerate(CHUNK_J):
                    clen = CHUNK_LEN[ci]
                    cbase = CHUNK_BASE[ci]
                    ps = {n0: scp.tile([128, 1024], f32, tag="sc", name="ps0"),
                          n1: scp.tile([128, 1024], f32, tag="sc", name="ps1")}
                    for j in js:
                        base = G[j] - cbase
                        for lo, hi in _bank_pieces(base, base + S - 128 * j):
                            s1a = 128 * j + (lo - base)
                            s1b = 128 * j + (hi - base)
                            diag = (lo == base)
                            if cfg.get("paired_scores"):
                                # concurrent row-tiles: head n at partition
                                # half 64*(n%2) -> tile (0,0) / (64,0)
                                for n in (n0, n1):
                                    po = 64 * (n % 2)
                                    nc.tensor.matmul(
                                        ps[n][:, lo:hi],
                                        lhsT=kT_z[po:po + 64, n,
                                                  128 * j:128 * (j + 1)],
                                        rhs=qT_m[po:po + 64, m, s1a:s1b],
                                        start=True, stop=not diag)
                            else:
                                for n in (n0, n1):
                                    nc.tensor.matmul(
                                        ps[n][:, lo:hi],
                                        lhsT=kT_z[:, n, 128 * j:128 * (j + 1)],
                                        rhs=qT_m[:, m, s1a:s1b],
                                        start=True, stop=not diag)
                            if diag:
                                for n in (n0, n1):
                                    nc.tensor.matmul(
                                        ps[n][:, base:base + 128],
                                        lhsT=tri, rhs=negeye,
                                        start=False, stop=True)
                    for n in (n0, n1):
                        if n in dve_heads:
                            dst = eT[n][:, cbase:cbase + clen].bitcast(i16)
                            nc.vector.tensor_scalar(dst, ps[n][:, 0:clen],
                                                    SCH_A, SCH_B, Op.mult, Op.add)
                        else:
                            nc.scalar.activation(eT[n][:, cbase:cbase + clen],
                                                 ps[n][:, 0:clen], AF.Exp, scale=SCALE)

                    if stop_after == "scores":
                        continue
                    # incremental PV: issue pieces for every j in this chunk
                    for j in js:
                        for n in (n0, n1):
                            for c in range(2):
                                if j > 4 * c + 3:
                                    continue
                                if j == 0:
                                    pvt[n, c] = pvp.tile([128, 512], f32, tag="pv",
                                                         name=f"pv{n % 2}{c}")
                                s1a = max(512 * c, 128 * j)
                                s1b = 512 * (c + 1)
                                last = (c == 0 and j == 3) or (c == 1 and j == 4)
                                nc.tensor.matmul(
                                    pvt[n, c][0:65, s1a - 512 * c:512],
                                    lhsT=v_aug[:, j, n, :],
                                    rhs=eT[n][:, G[j] + (s1a - 128 * j):G[j] + (s1b - 128 * j)],
                                    start=(j == 0), stop=last)
                                if last:
                                    nc.vector.tensor_copy(
                                        ul[:, 2 * c + (n - n0), :], pvt[n, c][0:65, :])
                    if stop_after is None:
                        if ci == 3:
                            epilogue_c(0)
                        elif ci == 4:
                            epilogue_c(1)

                if stop_after == "scores":
                    fin1 = epi.tile([128, 512], f32, tag="fin1")
                    for n in (n0, n1):
                        nc.vector.tensor_copy(fin1, eT[n][:, 0:512])
                        nc.sync.dma_start(
                            out=out_d[64 * (n // 2):64 * (n // 2) + 128,
                                      512 * (n % 2):512 * (n % 2) + 512],
                            in_=fin1)
                    continue

    nc.compile()
    return nc


_CACHE = {}


def _get_program(cfg_key=None):
    key = cfg_key or "default"
    if key not in _CACHE:
        _CACHE[key] = build_program(CFG)
    return _CACHE[key]


def make_in_map(inp, b):
    """Per-core input dict for batch b (weights host-transposed/cast to bf16;
    biases host-adjusted for the elu(x)+1 formulation)."""
    import ml_dtypes
    wt = np.dtype(ml_dtypes.bfloat16)
    m = {
        "query": np.ascontiguousarray(inp["query"][b].reshape(C, S)).astype(wt),
        "key": np.ascontiguousarray(inp["key"][b].reshape(C, S)).astype(wt),
    }
    for p in ("q", "k", "v"):
        w1 = inp[f"{p}_gr_w1"]
        w2 = inp[f"{p}_gr_w2"]
        m[f"{p}_wcat"] = np.ascontiguousarray(np.concatenate(
            [w1.T, w2.T, inp[f"{p}_nin_w"].T], axis=1)).astype(wt)
        b1_eff = inp[f"{p}_gr_b1"] - w1.sum(axis=1)
        b2_eff = inp[f"{p}_gr_b2"] - w2.sum(axis=1)
        m[f"{p}_bcat"] = np.concatenate(
            [b1_eff, 0.5 * b2_eff[:C], 0.5 * b2_eff[C:],
             b1_eff + 1.0]).astype(np.float32)
    return m


def kernel(**inputs):
    from concourse.bass_utils import run_bass_kernel_spmd

    nc = _get_program()
    inp = {k: np.asarray(v, dtype=np.float32) for k, v in inputs.items()}

    in_maps = [make_in_map(inp, b) for b in range(N_CORES)]

    trace = bool(int(os.environ.get("BASS_KERNEL_TRACE", "0")))
    res = run_bass_kernel_spmd(nc, in_maps, core_ids=list(range(N_CORES)),
                               trace=trace)
    LAST_RUN["exec_time_ns"] = getattr(res, "exec_time_ns", None)
    LAST_RUN["results"] = res
    out = np.stack([res.results[i]["out"].reshape(D, 32, 32)
                    for i in range(N_CORES)])
    return out.astype(np.float32)


LAST_RUN = {}


if __name__ == "__main__":
    nc = build_program()
    print("compiled OK")
